# revision 41
# baseline (speedup 1.0000x reference)
"""GraphSAGE (mean) 3-layer encoder on 8 Trainium2 NeuronCores.

Strategy (graph/data parallel, per sharding hint):
  - Nodes sharded contiguously across 8 cores (12500/core, padded to
    12544 = 98*128 slots); per-core nodes dealt round-robin by in-degree
    into 4 slot QUARTERS (24/24/25/25 blocks) so each quarter carries
    ~equal edge work.
  - Global feature table is QUARTER-major: (quarter, core, slot).  Each
    quarter of a core's shard AllGathers into a contiguous table range,
    and the 4 collectives per layer are issued incrementally as each
    quarter's blocks finish, overlapping with remaining compute.  Edge
    gathers of bucket b (src table quarter b) depend only on quarter-b's
    collective, so next-layer gathers start mid-previous-layer.
  - Edges routed by dst core on the host.  Per 128-slot dst block, edges
    grouped by src bucket (=table quarter, <=25600 rows so dma_gather's
    int16 indices reach every row) and packed densely into per-(group,
    bucket) segments using shared per-(block,bucket) slot allocations
    (max edge count over cores).  Groups are 3-4 blocks balanced within
    a quarter.
  - Per layer on device: dma_gather of src rows (bf16) -> 0/1 one-hot
    selectors built 8 columns per DVE tensor_tensor -> PE matmul
    accumulates SUM-aggregated neighborhoods feature-major into a shared
    per-group PSUM tile -> fp32r dense matmuls (self/neigh) double-
    buffered in PSUM -> batched group-level post: transposes to
    node-major, 1/deg scale + add + ReLU + L2 norm + residual with
    group-wide DVE/ACT ops -> bf16 h stored via HWDGE (sync) so the
    GpSimd/Pool engine only runs gather descriptor generation.
  - Layer 0 aggregates from the 5k vocab via per-group count matrices
    (fp8) using DoubleRow fp8 matmuls (2 vocab k-tiles per pass) with
    wide moving operands: no gathers and ~3x less PE time; its quarter
    collectives interleave with the first layer-1 gather prefetches.
  - Next-layer bucket-0/1/2 gather prefetches for the first two groups
    are hoisted into the previous layer's last iterations (ahead of its
    Q3 store/collective in Pool program order) so they drain during the
    tail; gather-ring WAR safety requires bufs=10 and the exact issue
    order b3(gi+1) -> b012(gi+3) (see the allocation-index analysis in
    comments).  Norm uses sqrt(ss + 1e-24) on ACT (the DVE
    tensor_scalar_max on small tiles costs ~6.6us/op and is avoided).
"""

import math
import sys

import numpy as np
import ml_dtypes

for _p in ("/opt/trn_rl_repo", "/root/.axon_site/_ro/trn_rl_repo"):
    if _p not in sys.path:
        sys.path.append(_p)

import concourse.bacc as bacc  # noqa: E402
import concourse.bass as bass  # noqa: E402
import concourse.mybir as mybir  # noqa: E402
import concourse.tile as tile  # noqa: E402
from concourse import bass_utils  # noqa: E402
from concourse.masks import make_identity  # noqa: E402

M = 8  # cores
D = 128
P = 128
NBUC = 4  # src buckets = table quarters
GRP = 4  # max dst blocks per group
NQ = 4  # SWDGE queues
VT = 40  # vocab tiles
VTH = VT // 2  # DoubleRow k-tile pairs
VP = VT * P  # padded vocab
EMB_SCALE = 64.0  # emb upscale for fp8 (undone via layer-0 1/deg scale)

LAST_EXEC_NS = None  # set by kernel() when _trace=True


def _host_prep(x, src, dst, n_nodes):
    N = n_nodes
    NPC = math.ceil(N / M)
    SLOTS = math.ceil(NPC / P) * P
    NBLK = SLOTS // P
    TBL = M * SLOTS

    # quarter split in blocks (as equal as possible)
    qb = NBLK // 4
    extra = NBLK - 4 * qb
    qsizes = [qb + (1 if i >= 4 - extra else 0) for i in range(4)]
    QBLK = np.concatenate([[0], np.cumsum(qsizes)]).astype(np.int64)
    QSTART = QBLK[:4] * P  # slot start of quarter
    QNS = np.diff(QBLK) * P  # slots per quarter
    QROWBASE = np.concatenate([[0], np.cumsum(M * QNS)])[:4]  # table row base
    BUCV = M * QNS  # bucket sizes
    assert BUCV.max() <= 32768

    x = np.asarray(x).astype(np.int64)
    src = np.asarray(src).astype(np.int64)
    dst = np.asarray(dst).astype(np.int64)

    deg = np.bincount(dst, minlength=N)
    core_of_node = np.minimum(np.arange(N) // NPC, M - 1)
    perm = np.empty(N, np.int64)
    for c in range(M):
        lo, hi = c * NPC, min((c + 1) * NPC, N)
        nodes = np.arange(lo, hi)
        order = np.argsort(deg[nodes], kind="stable")
        nn_ = len(nodes)
        # deal ranked nodes round-robin into quarters (degree balance);
        # small quarters (cap qsizes[q]*P) fill first, tail goes to big ones
        caps = QNS.copy()
        r = np.arange(nn_)
        even = 4 * caps.min()
        q = np.where(r < even, r % 4, 0)
        within = np.where(r < even, r // 4, 0)
        if nn_ > even:
            rt = r[even:] - even
            # remaining capacity only in quarters with caps > caps.min()
            bigq = np.where(caps > caps.min())[0]
            q[even:] = bigq[rt % len(bigq)]
            within[even:] = caps.min() + rt // len(bigq)
        slot = QSTART[q] + within
        perm[nodes[order]] = slot
    # quarter-major global table row
    qofs = np.searchsorted(QSTART, perm, side="right") - 1
    grow = QROWBASE[qofs] + core_of_node * QNS[qofs] + (perm - QSTART[qofs])

    ecore = core_of_node[dst]
    cores_edges = []
    cnt_cjb = np.zeros((M, NBLK, NBUC), np.int64)
    for c in range(M):
        sel = ecore == c
        dslot = perm[dst[sel]]
        sg = grow[src[sel]]
        buc = np.searchsorted(QROWBASE, sg, side="right") - 1
        blk = dslot // P
        o = np.lexsort((dslot, buc, blk))
        dslot, sg, buc, blk = dslot[o], sg[o], buc[o], blk[o]
        cores_edges.append((dslot, sg, buc, blk))
        np.add.at(cnt_cjb[c], (blk, buc), 1)

    A_jb = cnt_cjb.max(axis=0)  # [NBLK, NBUC]
    for j in range(NBLK):
        if A_jb[j].sum() == 0:
            A_jb[j, 0] = 1  # zero-degree block still produces neigh=0

    # balance blocks into groups of <= GRP within each quarter
    C_j = A_jb.sum(axis=1)
    groups = []
    qgroup_end = []
    for q in range(4):
        blocks = list(range(QBLK[q], QBLK[q + 1]))
        ngroups = math.ceil(len(blocks) / GRP)
        order = sorted(blocks, key=lambda j: -C_j[j])
        gsum = np.zeros(ngroups, np.int64)
        gcnt = np.zeros(ngroups, np.int64)
        qgroups = [[] for _ in range(ngroups)]
        for j in order:
            cand = [g for g in range(ngroups) if gcnt[g] < GRP]
            g = min(cand, key=lambda t: gsum[t])
            qgroups[g].append(int(j))
            gsum[g] += C_j[j]
            gcnt[g] += 1
        groups.extend(sorted(g) for g in qgroups)
        qgroup_end.append(len(groups) - 1)
    NG = len(groups)

    # stream layout: for g, for b: packed segment of the group's blocks
    off_jb = np.zeros((NBLK, NBUC), np.int64)
    calls = []  # per group: list of (b, ch0, nch, ni)
    seg_of = {}
    pos = 0
    for g in groups:
        gc = []
        for b in range(NBUC):
            seg = 0
            for j in g:
                seg_of[(j, b)] = (pos, seg)
                off_jb[j, b] = pos * P + seg
                seg += int(A_jb[j, b])
            if seg > 0:
                nch = math.ceil(seg / P)
                gc.append((b, pos, nch, seg))
                pos += nch
        calls.append(gc)
    NCH = pos
    NIDX = NCH * P

    # (chunk, block) matmul pairs and one-hot columns, block-major 8-aligned
    blockmm = {j: [] for j in range(NBLK)}
    dcol0 = np.zeros(NBLK, np.int64)
    dpos = 0
    for j in range(NBLK):
        dcol0[j] = dpos
        i = 0
        for b in range(NBUC):
            if A_jb[j, b] == 0:
                continue
            ch0, so = seg_of[(j, b)]
            lo = ch0 * P + so
            hi = lo + int(A_jb[j, b])
            c_lo, c_hi = lo // P, (hi - 1) // P
            for sp in range(c_lo, c_hi + 1):
                blockmm[j].append((b, sp, dpos + i))
                i += 1
        dpos += 8 * math.ceil(i / 8)
    NCHD = dpos

    # layer-0 c8 sub-tiles: per group, 2 subs of <=2 blocks (DoubleRow
    # interleaved layout [P, VTH, 2, 2*P])
    subblocks = []  # per sub: list of blocks (len<=2)
    sub_of_group = []  # per group: (sub0, sub1)
    for g in groups:
        s0 = len(subblocks)
        subblocks.append(g[0:2])
        subblocks.append(g[2:4])
        sub_of_group.append((s0, s0 + 1))
    NSUB = len(subblocks)

    per_core = []
    for c in range(M):
        dslot, sg, buc, blk = cores_edges[c]
        sel = ecore == c
        cnt = np.zeros((VP, SLOTS), np.int16)
        np.add.at(cnt, (x[src[sel]], perm[dst[sel]]), 1)
        # [VTH, 2, P, NBLK, P] view: vocab row (2*th+i)*P + p
        cnt5 = cnt.reshape(VTH, 2, P, NBLK, P)
        c8 = np.zeros((NSUB, P, VTH, 2, 2 * P), np.float32)
        for s, bl in enumerate(subblocks):
            for w, j in enumerate(bl):
                c8[s, :, :, :, w * P : (w + 1) * P] = cnt5[:, :, :, j, :].transpose(
                    2, 0, 1, 3
                )
        c8 = c8.reshape(NSUB, P, VTH * 2 * 2 * P).astype(ml_dtypes.float8_e4m3fn)

        # rank within (block, bucket)
        flat = (blk * NBUC + buc).astype(np.int64)
        cnts = cnt_cjb[c].reshape(-1)
        st = np.zeros(NBLK * NBUC, np.int64)
        st[1:] = np.cumsum(cnts)[:-1]
        rank = np.arange(len(dslot)) - st[flat]
        spos = off_jb[blk, buc] + rank
        idxs = np.zeros(NIDX, np.int16)
        idxs[spos] = (sg - QROWBASE[buc]).astype(np.int16)
        # NOTE: do NOT mark trailing pad slots as -1 (the gather kernel's
        # trailing-negative trim) — it deadlocks the device even when
        # num_idxs stays >= 1 after trimming (tested twice).

        dstloc = np.full((P, NCHD), 255.0, ml_dtypes.bfloat16)
        ech = spos // P
        first_ch = off_jb[blk, buc] // P
        # dc: walk each (j,b) range; dcol0[j] + offset within block's list
        # recompute per-edge dst col via blockmm structure
        dc = np.zeros(len(dslot), np.int64)
        # per (j, b): columns assigned consecutively from (j,b) range start
        colbase = {}
        for j in range(NBLK):
            for (b, sp, dcol) in blockmm[j]:
                colbase.setdefault((j, b), dcol)
        cb_lookup = np.zeros((NBLK, NBUC), np.int64)
        fc_lookup = np.zeros((NBLK, NBUC), np.int64)
        for (j, b), dcol in colbase.items():
            cb_lookup[j, b] = dcol
            fc_lookup[j, b] = off_jb[j, b] // P
        dc = cb_lookup[blk, buc] + (ech - fc_lookup[blk, buc])
        dstloc[spos % P, dc] = (dslot % P).astype(np.float32)

        idx16 = idxs.reshape(NIDX // 16, 16).T.copy()
        idx_full = np.tile(idx16, (8, 1))

        lo = c * NPC
        invd = 1.0 / np.maximum(deg, 1.0)
        nodes = np.arange(lo, min((c + 1) * NPC, N))
        node_of_slot = np.full(SLOTS, -1, np.int64)
        node_of_slot[perm[nodes]] = nodes
        invd_slot = np.ones(SLOTS, np.float32)
        real = node_of_slot >= 0
        invd_slot[real] = invd[node_of_slot[real]].astype(np.float32)
        # grouped: invd_g[p, g*GRP + bi] = invd_slot[j*P + p]
        invd_g = np.ones((P, NG * GRP), np.float32)
        for g, bl in enumerate(groups):
            for bi, j in enumerate(bl):
                invd_g[:, g * GRP + bi] = invd_slot[j * P : (j + 1) * P]

        x_slot = np.zeros(SLOTS, np.int64)
        x_slot[perm[nodes]] = x[nodes]
        xg = x_slot.astype(np.int16)
        xg16 = np.tile(xg.reshape(SLOTS // 16, 16).T.copy(), (8, 1))

        per_core.append(
            {
                "gidx": idx_full,
                "dstloc": dstloc,
                "invd": invd_g,
                "invd0": invd_g / EMB_SCALE,
                "xgidx": xg16,
                "c8": c8,
            }
        )

    gslot = core_of_node * SLOTS + perm

    meta = {
        "NPC": NPC,
        "SLOTS": SLOTS,
        "NBLK": NBLK,
        "TBL": TBL,
        "QBLK": QBLK,
        "QSTART": QSTART,
        "QNS": QNS,
        "QROWBASE": QROWBASE,
        "BUCV": BUCV,
        "groups": groups,
        "qgroup_end": qgroup_end,
        "calls": calls,
        "blockmm": blockmm,
        "dcol0": dcol0,
        "NCH": NCH,
        "NCHD": NCHD,
        "NIDX": NIDX,
        "NG": NG,
        "NSUB": NSUB,
        "sub_of_group": sub_of_group,
        "gslot": gslot,
    }
    return per_core, meta


def _build_program(meta, V, L, single_core=False):
    SLOTS, NBLK, TBL = meta["SLOTS"], meta["NBLK"], meta["TBL"]
    QBLK, QSTART, QNS = meta["QBLK"], meta["QSTART"], meta["QNS"]
    QROWBASE, BUCV = meta["QROWBASE"], meta["BUCV"]
    groups, qgroup_end = meta["groups"], meta["qgroup_end"]
    calls, blockmm, dcol0 = meta["calls"], meta["blockmm"], meta["dcol0"]
    NCH, NCHD, NIDX, NG = meta["NCH"], meta["NCHD"], meta["NIDX"], meta["NG"]
    NSUB, sub_of_group = meta["NSUB"], meta["sub_of_group"]
    CBMAX = max(nch for gc in calls for (_, _, nch, _) in gc)

    f32, f32r, bf16 = mybir.dt.float32, mybir.dt.float32r, mybir.dt.bfloat16
    i16, f8 = mybir.dt.int16, mybir.dt.float8e4

    nc = bacc.Bacc(
        "TRN2",
        target_bir_lowering=False,
        debug=False,
        enable_asserts=False,
        num_devices=1 if single_core else M,
        num_swdge_queues=NQ,
    )

    gidx_d = nc.dram_tensor("gidx", [P, NIDX // 16], i16, kind="ExternalInput")
    dstloc_d = nc.dram_tensor("dstloc", [P, NCHD], bf16, kind="ExternalInput")
    invd_d = nc.dram_tensor("invd", [P, NG * GRP], f32, kind="ExternalInput")
    invd0_d = nc.dram_tensor("invd0", [P, NG * GRP], f32, kind="ExternalInput")
    emb8_d = nc.dram_tensor("emb8", [P, VT * D], f8, kind="ExternalInput")
    c8_d = nc.dram_tensor("c8", [NSUB, P, VTH * 2 * 2 * P], f8, kind="ExternalInput")
    xgidx_d = nc.dram_tensor("xgidx", [P, SLOTS // 16], i16, kind="ExternalInput")
    emb16_d = nc.dram_tensor("emb16", [V, D], bf16, kind="ExternalInput")
    ws_d = nc.dram_tensor("ws", [L, D, D], f32, kind="ExternalInput")
    wn_d = nc.dram_tensor("wn", [L, D, D], f32, kind="ExternalInput")
    bias_d = nc.dram_tensor("bias", [L, D], f32, kind="ExternalInput")
    hout_d = nc.dram_tensor("hout", [SLOTS, D], bf16, kind="ExternalOutput")

    h_shard = nc.dram_tensor("h_shard", [SLOTS, D], bf16, kind="Internal")
    h_full_t = [
        nc.dram_tensor(
            f"h_full{t}", [TBL, D], bf16, kind="Internal", addr_space="Shared"
        )
        for t in range(2)
    ]

    rg = [list(range(M))]
    qrr = [0]

    with tile.TileContext(nc) as tc:
        with (
            tc.tile_pool(name="const", bufs=1) as cpool,
            tc.tile_pool(name="state", bufs=1) as spool,
            tc.tile_pool(name="gath", bufs=12) as gpool,
            tc.tile_pool(name="oh", bufs=5) as ohpool,
            tc.tile_pool(name="c8", bufs=2) as c8pool,
            tc.tile_pool(name="fm", bufs=2) as fmpool,
            tc.tile_pool(name="small", bufs=1) as smpool,
            tc.tile_pool(name="ps_a", bufs=2, space="PSUM") as ps_a,
            tc.tile_pool(name="ps_t", bufs=2, space="PSUM") as ps_t,
            tc.tile_pool(name="ps_d", bufs=2, space="PSUM") as ps_d,
        ):
            # ---- constants ----
            ident_f = cpool.tile([P, P], f32, tag="ident_f")
            make_identity(nc, ident_f[:])
            ident_h = cpool.tile([P, P], bf16, tag="ident_h")
            make_identity(nc, ident_h[:])
            eps_sb = cpool.tile([P, 1], f32, tag="eps")
            nc.vector.memset(eps_sb[:], 1e-24)
            iota8 = cpool.tile([P, 8 * P], bf16, tag="iota8")
            nc.gpsimd.iota(
                iota8[:].rearrange("p (r c) -> p r c", c=P),
                pattern=[[0, 8], [1, P]],
                base=0,
                channel_multiplier=0,
                allow_small_or_imprecise_dtypes=True,
            )

            gidx_sb = cpool.tile([P, NIDX // 16], i16, tag="gidx")
            nc.sync.dma_start(gidx_sb[:], gidx_d[:, :])
            dstloc_sb = cpool.tile([P, NCHD], bf16, tag="dstloc")
            nc.sync.dma_start(dstloc_sb[:], dstloc_d[:, :])
            invd_sb = cpool.tile([P, NG * GRP], f32, tag="invd")
            nc.sync.dma_start(invd_sb[:], invd_d[:, :])
            invd0_sb = cpool.tile([P, NG * GRP], f32, tag="invd0")
            nc.sync.dma_start(invd0_sb[:], invd0_d[:, :])
            emb8_sb = cpool.tile([P, VT * D], f8, tag="emb8")
            nc.sync.dma_start(emb8_sb[:], emb8_d[:, :])
            xg_sb = cpool.tile([P, SLOTS // 16], i16, tag="xgidx")
            nc.sync.dma_start(xg_sb[:], xgidx_d[:, :])

            w_sb = []
            for l in range(L):
                wsf = cpool.tile([P, D], f32, tag=f"wsf{l}")
                wnf = cpool.tile([P, D], f32, tag=f"wnf{l}")
                nc.sync.dma_start(wsf[:], ws_d[l, :, :])
                nc.sync.dma_start(wnf[:], wn_d[l, :, :])
                ws = cpool.tile([P, D], f32r, tag=f"ws{l}")
                wn = cpool.tile([P, D], f32r, tag=f"wn{l}")
                nc.scalar.copy(ws[:], wsf[:])
                nc.scalar.copy(wn[:], wnf[:])
                w_sb.append((ws, wn))
            b_sb = cpool.tile([P, L], f32, tag="bias")
            for l in range(L):
                nc.sync.dma_start(b_sb[:, l : l + 1], bias_d[l, :, None])

            # ---- embedding lookup: 4 dma_gather calls, one per queue ----
            e_sb = spool.tile([P, NBLK * D], bf16, tag="e")
            ev = e_sb[:].rearrange("p (j f) -> p j f", f=D)
            jsplit = [0, 25, 50, 75, NBLK]
            for qi in range(4):
                j0, j1 = jsplit[qi], jsplit[qi + 1]
                ni = (j1 - j0) * P
                nc.gpsimd.dma_gather(
                    ev[:, j0:j1, :],
                    emb16_d[:, :],
                    xg_sb[:, j0 * 8 : j1 * 8],
                    ni,
                    ni,
                    D,
                    single_packet=False,
                    queue_num=qi % NQ,
                )

            h_sb = spool.tile([P, NBLK * D], bf16, tag="h")

            # zero-init gather ring buffers (NaN * 0 = NaN safety)
            for _ in range(12):
                gz = gpool.tile([P, CBMAX, D], bf16, tag="gath")
                nc.vector.memset(gz[:, :, :], 0.0)

            shard_v = h_shard.ap().rearrange("(j p) f -> p j f", p=P)

            def store_q(q, tab):
                j0, j1 = int(QBLK[q]), int(QBLK[q + 1])
                sv = h_sb[:, j0 * D : j1 * D].rearrange("p (j f) -> p j f", f=D)
                # scalar (ACT) HWDGE ring: the sync ring is clogged with
                # WAR-throttled c8 loads in layer 0, which would delay the
                # store and hence the collective trigger by ~100us.  The
                # store has no WAR wait, so it can't head-of-line block ACT.
                nc.scalar.dma_start(out=shard_v[:, j0:j1, :], in_=sv)
                if single_core:
                    return
                nc.gpsimd.collective_compute(
                    "AllGather",
                    mybir.AluOpType.bypass,
                    replica_groups=rg,
                    ins=[h_shard[int(QSTART[q]) : int(QSTART[q] + QNS[q]), :]],
                    outs=[
                        tab[
                            int(QROWBASE[q]) : int(QROWBASE[q] + M * QNS[q]), :
                        ]
                    ],
                )

            qend_of = {qgroup_end[q]: q for q in range(4)}

            gtiles = {}  # (layer, gi) -> {bucket: (tile, ch0)}

            def issue(li, gi, buckets):
                tab_in = h_full_t[li % 2]
                gt_map = gtiles.setdefault((li, gi), {})
                for (b, ch0, nch, ni) in calls[gi]:
                    if b not in buckets:
                        continue
                    gt = gpool.tile([P, CBMAX, D], bf16, tag="gath")
                    nc.gpsimd.dma_gather(
                        gt[:, 0:nch, :],
                        tab_in[int(QROWBASE[b]) : int(QROWBASE[b] + BUCV[b]), :],
                        gidx_sb[:, ch0 * 8 : (ch0 + nch) * 8],
                        ni,
                        ni,
                        D,
                        single_packet=False,
                        queue_num=qrr[0] % NQ,
                    )
                    qrr[0] += 1
                    gt_map[b] = (gt, ch0)

            # ---- layers ----
            for l in range(L):
                cur = e_sb if l == 0 else h_sb
                h_full = h_full_t[l % 2]
                tab_out = h_full_t[(l + 1) % 2]
                ws, wn = w_sb[l]

                if l > 0:
                    # groups 0/1 b0/b1 were hoisted into the previous
                    # layer; their b2 calls run here (they SEQ-wait on the
                    # Q2 collective, so hoisting them would block Pool)
                    issue(l, 0, (2,))
                    issue(l, 1, (2,))
                    issue(l, 2, (0, 1, 2))
                    issue(l, 0, (3,))
                    issue(l, 1, (3,))

                post = None
                for gi, grp in enumerate(groups):
                    gw = len(grp)
                    if (gi - 1) in qend_of and post is not None:
                        # run the deferred post + quarter store/collective
                        # BEFORE this iteration's gather issues so the
                        # trigger isn't queued behind ring-stalled DGE
                        post()
                        post = None
                        if l < L - 1:
                            store_q(qend_of[gi - 1], tab_out)
                    if l > 0:
                        # order matters for gather-ring WAR safety:
                        # b3(gi+2) must precede b012(gi+3); the 2-group
                        # bucket-3 lead hides the gather latency that
                        # otherwise stalls every group's chain tail
                        if gi + 2 < NG:
                            issue(l, gi + 2, (3,))
                        if gi + 3 < NG:
                            issue(l, gi + 3, (0, 1, 2))
                    pa4 = ps_a.tile([P, GRP * P], f32, tag="pa")
                    if l == 0:
                        # layer 0: neigh-sum = emb8^T @ count-matrix via
                        # fp8 DoubleRow (2 vocab k-tiles per matmul)
                        ev8 = emb8_sb[:].rearrange(
                            "p (t i f) -> p t i f", t=VTH, i=2
                        )
                        for si in range(2):
                            sidx = sub_of_group[gi][si]
                            cs = c8pool.tile(
                                [P, VTH * 2 * 2 * P], f8, tag="c8"
                            )
                            # sync ring only (the ACT ring carries the
                            # quarter stores); split each tile in quarters
                            # so the matmul chain starts on the first one
                            QB = VTH * 2 * 2 * P // 4
                            for ci in range(4):
                                nc.sync.dma_start(
                                    cs[:, ci * QB : (ci + 1) * QB],
                                    c8_d[sidx, :, ci * QB : (ci + 1) * QB],
                                )
                            csv = cs[:].rearrange(
                                "p (t i w) -> p t i w", t=VTH, i=2
                            )
                            for th in range(VTH):
                                nc.tensor.matmul(
                                    pa4[:, si * 2 * P : (si + 1) * 2 * P],
                                    ev8[:, th, :, :],
                                    csv[:, th, :, :],
                                    start=(th == 0),
                                    stop=(th == VTH - 1),
                                    perf_mode=mybir.MatmulPerfMode.DoubleRow,
                                )
                    else:
                        for bi, j in enumerate(grp):
                            mms = blockmm[j]
                            nmm = len(mms)
                            noct = (nmm + 7) // 8
                            d0 = int(dcol0[j])
                            ohqs = []
                            for t in range(noct):
                                ohq = ohpool.tile([P, 8 * P], bf16, tag="oh")
                                s = d0 + 8 * t
                                nc.vector.tensor_tensor(
                                    out=ohq[:].rearrange(
                                        "p (r c) -> p r c", c=P
                                    ),
                                    in0=dstloc_sb[
                                        :, s : s + 8
                                    ].to_broadcast([P, 8, P]),
                                    in1=iota8[:].rearrange(
                                        "p (r c) -> p r c", c=P
                                    ),
                                    op=mybir.AluOpType.is_equal,
                                )
                                ohqs.append(ohq)
                            for ci, (b, sp, dc) in enumerate(mms):
                                gt, ch0 = gtiles[(l, gi)][b]
                                q8, r8 = divmod(dc - d0, 8)
                                nc.tensor.matmul(
                                    pa4[:, bi * P : (bi + 1) * P],
                                    gt[:, sp - ch0, :],
                                    ohqs[q8][:, r8 * P : (r8 + 1) * P],
                                    start=(ci == 0),
                                    stop=(ci == nmm - 1),
                                )
                    nfm = fmpool.tile([P, GRP * D], f32r, tag="nfm", bufs=1)
                    # layer 0 is ACT-heavy; route its PSUM copies to DVE
                    cp_eng = nc.vector.tensor_copy if l == 0 else nc.scalar.copy
                    cp_eng(nfm[:, 0 : gw * D], pa4[:, 0 : gw * P])
                    # self path: transpose cur blocks to feature-major
                    pt4 = ps_t.tile([P, GRP * P], bf16, tag="pt")
                    for bi, j in enumerate(grp):
                        nc.tensor.transpose(
                            pt4[:, bi * P : (bi + 1) * P],
                            cur[:, j * D : (j + 1) * D],
                            ident_h[:],
                        )
                    hfm = fmpool.tile([P, GRP * D], f32r, tag="hfm", bufs=1)
                    nc.scalar.copy(hfm[:, 0 : gw * D], pt4[:, 0 : gw * P])

                    d_ps = ps_d.tile([P, 2 * GRP * D], f32, tag="d")
                    pdS = d_ps[:, 0 : GRP * D]
                    pdN = d_ps[:, GRP * D : 2 * GRP * D]
                    nc.tensor.matmul(
                        pdS[:, 0 : gw * D],
                        ws[:],
                        hfm[:, 0 : gw * D],
                        start=True,
                        stop=True,
                    )
                    nc.tensor.matmul(
                        pdN[:, 0 : gw * D],
                        wn[:],
                        nfm[:, 0 : gw * D],
                        start=True,
                        stop=True,
                    )
                    hbias = fmpool.tile([P, GRP * D], f32, tag="hbias")
                    nc.scalar.activation(
                        hbias[:, 0 : gw * D],
                        pdS[:, 0 : gw * D],
                        mybir.ActivationFunctionType.Identity,
                        bias=b_sb[:, l : l + 1],
                    )
                    nden = fmpool.tile([P, GRP * D], f32, tag="nden")
                    cp_eng(nden[:, 0 : gw * D], pdN[:, 0 : gw * D])

                    def make_post(gi=gi, grp=grp, hbias=hbias, nden=nden, l=l):
                        def run():
                            gw = len(grp)
                            nm4 = ps_d.tile([P, 2 * GRP * D], f32, tag="d")
                            pnS = nm4[:, 0 : GRP * P]
                            pnN = nm4[:, GRP * P : 2 * GRP * P]
                            for bi in range(gw):
                                nc.tensor.transpose(
                                    pnS[:, bi * P : (bi + 1) * P],
                                    hbias[:, bi * D : (bi + 1) * D],
                                    ident_f[:],
                                )
                                nc.tensor.transpose(
                                    pnN[:, bi * P : (bi + 1) * P],
                                    nden[:, bi * D : (bi + 1) * D],
                                    ident_f[:],
                                )
                            inv_l = invd0_sb if l == 0 else invd_sb
                            tn4 = smpool.tile([P, GRP * P], f32, tag="tn4")
                            for bi in range(gw):
                                nc.scalar.activation(
                                    tn4[:, bi * P : (bi + 1) * P],
                                    pnN[:, bi * P : (bi + 1) * P],
                                    mybir.ActivationFunctionType.Identity,
                                    scale=inv_l[
                                        :, gi * GRP + bi : gi * GRP + bi + 1
                                    ],
                                )
                            hp4 = smpool.tile([P, GRP * P], f32, tag="hp4")
                            nc.vector.tensor_tensor(
                                out=hp4[:, 0 : gw * P],
                                in0=pnS[:, 0 : gw * P],
                                in1=tn4[:, 0 : gw * P],
                                op=mybir.AluOpType.add,
                            )
                            # relu into tn4 (dead after the add)
                            nc.scalar.activation(
                                tn4[:, 0 : gw * P],
                                hp4[:, 0 : gw * P],
                                mybir.ActivationFunctionType.Relu,
                            )
                            ss4 = smpool.tile([P, GRP], f32, tag="ss4")
                            for bi in range(gw):
                                # squares land in hp4 (dead); only the
                                # accumulated sum-of-squares is consumed
                                nc.scalar.activation(
                                    hp4[:, bi * P : (bi + 1) * P],
                                    tn4[:, bi * P : (bi + 1) * P],
                                    mybir.ActivationFunctionType.Square,
                                    accum_out=ss4[:, bi : bi + 1],
                                )
                            nrm4 = smpool.tile([P, GRP], f32, tag="nrm4")
                            # sqrt(ss + 1e-24) == the 1e-12 norm floor
                            nc.scalar.activation(
                                nrm4[:, 0:gw],
                                ss4[:, 0:gw],
                                mybir.ActivationFunctionType.Sqrt,
                                bias=eps_sb[:, 0:1],
                            )
                            inv4 = smpool.tile([P, GRP], f32, tag="inv4")
                            nc.vector.reciprocal(inv4[:, 0:gw], nrm4[:, 0:gw])
                            ht4 = smpool.tile([P, GRP * P], f32, tag="ht4")
                            for bi in range(gw):
                                nc.scalar.activation(
                                    ht4[:, bi * P : (bi + 1) * P],
                                    tn4[:, bi * P : (bi + 1) * P],
                                    mybir.ActivationFunctionType.Identity,
                                    scale=inv4[:, bi : bi + 1],
                                )
                            for bi, j in enumerate(grp):
                                nc.vector.tensor_tensor(
                                    out=h_sb[:, j * D : (j + 1) * D],
                                    in0=ht4[:, bi * P : (bi + 1) * P],
                                    in1=e_sb[:, j * D : (j + 1) * D],
                                    op=mybir.AluOpType.add,
                                )
                        return run

                    if post is not None:
                        post()
                    post = make_post()
                post()
                if l < L - 1:
                    store_q(3, tab_out)
                    # hoist next layer's first b0/b1 prefetches here: after
                    # the Q3 collective trigger (so it fires promptly) and
                    # limited to buckets whose collectives are long done so
                    # the hoist never SEQ-blocks the Pool queue
                    issue(l + 1, 0, (0, 1))
                    issue(l + 1, 1, (0, 1))

            hout_v = hout_d.ap().rearrange("(j p) f -> p j f", p=P)
            h_v = h_sb[:].rearrange("p (j f) -> p j f", f=D)
            nc.sync.dma_start(hout_v, h_v)

    nc.compile()
    return nc


def kernel(x, src, dst, emb, Ws, Wn, b, _trace=False):
    x = np.asarray(x)
    src = np.asarray(src)
    dst = np.asarray(dst)
    emb = np.ascontiguousarray(np.asarray(emb, dtype=np.float32))
    Ws = np.ascontiguousarray(np.asarray(Ws, dtype=np.float32))
    Wn = np.ascontiguousarray(np.asarray(Wn, dtype=np.float32))
    b = np.ascontiguousarray(np.asarray(b, dtype=np.float32))
    N = x.shape[0]
    V, _ = emb.shape
    L = Ws.shape[0]

    per_core, meta = _host_prep(x, src, dst, N)
    nc = _build_program(meta, V, L)

    # emb, upscaled for fp8, DoubleRow-interleaved SBUF layout:
    # emb8s[p, (th*2 + i)*D + f] = (emb * EMB_SCALE)[(2*th+i)*P + p, f]
    embp = np.zeros((VP, D), np.float32)
    embp[:V] = emb * EMB_SCALE
    emb8s = np.ascontiguousarray(
        embp.reshape(VT, P, D).transpose(1, 0, 2).reshape(P, VT * D)
    ).astype(ml_dtypes.float8_e4m3fn)
    emb16 = np.ascontiguousarray(emb.astype(ml_dtypes.bfloat16))

    in_maps = []
    for c in range(M):
        pc = per_core[c]
        in_maps.append(
            {
                "gidx": np.ascontiguousarray(pc["gidx"]),
                "dstloc": np.ascontiguousarray(pc["dstloc"]),
                "invd": np.ascontiguousarray(pc["invd"]),
                "invd0": np.ascontiguousarray(pc["invd0"]),
                "xgidx": np.ascontiguousarray(pc["xgidx"]),
                "c8": np.ascontiguousarray(pc["c8"]),
                "emb8": emb8s,
                "emb16": emb16,
                "ws": Ws,
                "wn": Wn,
                "bias": b,
            }
        )

    res = bass_utils.run_bass_kernel_spmd(
        nc, in_maps, core_ids=list(range(M)), trace=_trace
    )
    global LAST_EXEC_NS
    LAST_EXEC_NS = res.exec_time_ns
    outs = [np.asarray(r["hout"], dtype=np.float32) for r in res.results]
    big = np.concatenate(outs, axis=0)
    return big[meta["gslot"]]


# revision 42
# speedup vs baseline: 1.0265x; 1.0265x over previous
"""GraphSAGE (mean) 3-layer encoder on 8 Trainium2 NeuronCores.

Strategy (graph/data parallel, per sharding hint):
  - Nodes sharded contiguously across 8 cores (12500/core, padded to
    12544 = 98*128 slots); per-core nodes dealt round-robin by in-degree
    into 4 slot QUARTERS (24/24/25/25 blocks) so each quarter carries
    ~equal edge work.
  - Global feature table is QUARTER-major: (quarter, core, slot).  Each
    quarter of a core's shard AllGathers into a contiguous table range,
    and the 4 collectives per layer are issued incrementally as each
    quarter's blocks finish, overlapping with remaining compute.  Edge
    gathers of bucket b (src table quarter b) depend only on quarter-b's
    collective, so next-layer gathers start mid-previous-layer.
  - Edges routed by dst core on the host.  Per 128-slot dst block, edges
    grouped by src bucket (=table quarter, <=25600 rows so dma_gather's
    int16 indices reach every row) and packed densely into per-(group,
    bucket) segments using shared per-(block,bucket) slot allocations
    (max edge count over cores).  Groups are 3-4 blocks balanced within
    a quarter.
  - Per layer on device: dma_gather of src rows (bf16) -> 0/1 one-hot
    selectors built 8 columns per DVE tensor_tensor -> PE matmul
    accumulates SUM-aggregated neighborhoods feature-major into a shared
    per-group PSUM tile -> fp32r dense matmuls (self/neigh) double-
    buffered in PSUM -> batched group-level post: transposes to
    node-major, 1/deg scale + add + ReLU + L2 norm + residual with
    group-wide DVE/ACT ops -> bf16 h stored via HWDGE (sync) so the
    GpSimd/Pool engine only runs gather descriptor generation.
  - Layer 0 aggregates from the 5k vocab via per-group count matrices
    (fp8) using DoubleRow fp8 matmuls (2 vocab k-tiles per pass) with
    wide moving operands: no gathers and ~3x less PE time; its quarter
    collectives interleave with the first layer-1 gather prefetches.
  - Next-layer bucket-0/1/2 gather prefetches for the first two groups
    are hoisted into the previous layer's last iterations (ahead of its
    Q3 store/collective in Pool program order) so they drain during the
    tail; gather-ring WAR safety requires bufs=10 and the exact issue
    order b3(gi+1) -> b012(gi+3) (see the allocation-index analysis in
    comments).  Norm uses sqrt(ss + 1e-24) on ACT (the DVE
    tensor_scalar_max on small tiles costs ~6.6us/op and is avoided).
"""

import math
import sys

import numpy as np
import ml_dtypes

for _p in ("/opt/trn_rl_repo", "/root/.axon_site/_ro/trn_rl_repo"):
    if _p not in sys.path:
        sys.path.append(_p)

import concourse.bacc as bacc  # noqa: E402
import concourse.bass as bass  # noqa: E402
import concourse.mybir as mybir  # noqa: E402
import concourse.tile as tile  # noqa: E402
from concourse import bass_utils  # noqa: E402
from concourse.masks import make_identity  # noqa: E402

M = 8  # cores
D = 128
P = 128
NBUC = 4  # src buckets = table quarters
GRP = 4  # max dst blocks per group
NQ = 4  # SWDGE queues
VT = 40  # vocab tiles
VTH = VT // 2  # DoubleRow k-tile pairs
VP = VT * P  # padded vocab
EMB_SCALE = 64.0  # emb upscale for fp8 (undone via layer-0 1/deg scale)

LAST_EXEC_NS = None  # set by kernel() when _trace=True


def _host_prep(x, src, dst, n_nodes):
    N = n_nodes
    NPC = math.ceil(N / M)
    SLOTS = math.ceil(NPC / P) * P
    NBLK = SLOTS // P
    TBL = M * SLOTS

    # quarter split in blocks (as equal as possible)
    qb = NBLK // 4
    extra = NBLK - 4 * qb
    qsizes = [qb + (1 if i >= 4 - extra else 0) for i in range(4)]
    QBLK = np.concatenate([[0], np.cumsum(qsizes)]).astype(np.int64)
    QSTART = QBLK[:4] * P  # slot start of quarter
    QNS = np.diff(QBLK) * P  # slots per quarter
    QROWBASE = np.concatenate([[0], np.cumsum(M * QNS)])[:4]  # table row base
    BUCV = M * QNS  # bucket sizes
    assert BUCV.max() <= 32768

    x = np.asarray(x).astype(np.int64)
    src = np.asarray(src).astype(np.int64)
    dst = np.asarray(dst).astype(np.int64)

    deg = np.bincount(dst, minlength=N)
    core_of_node = np.minimum(np.arange(N) // NPC, M - 1)
    perm = np.empty(N, np.int64)
    for c in range(M):
        lo, hi = c * NPC, min((c + 1) * NPC, N)
        nodes = np.arange(lo, hi)
        order = np.argsort(deg[nodes], kind="stable")
        nn_ = len(nodes)
        # deal ranked nodes round-robin into quarters (degree balance);
        # small quarters (cap qsizes[q]*P) fill first, tail goes to big ones
        caps = QNS.copy()
        r = np.arange(nn_)
        even = 4 * caps.min()
        q = np.where(r < even, r % 4, 0)
        within = np.where(r < even, r // 4, 0)
        if nn_ > even:
            rt = r[even:] - even
            # remaining capacity only in quarters with caps > caps.min()
            bigq = np.where(caps > caps.min())[0]
            q[even:] = bigq[rt % len(bigq)]
            within[even:] = caps.min() + rt // len(bigq)
        slot = QSTART[q] + within
        perm[nodes[order]] = slot
    # quarter-major global table row
    qofs = np.searchsorted(QSTART, perm, side="right") - 1
    grow = QROWBASE[qofs] + core_of_node * QNS[qofs] + (perm - QSTART[qofs])

    ecore = core_of_node[dst]
    cores_edges = []
    cnt_cjb = np.zeros((M, NBLK, NBUC), np.int64)
    for c in range(M):
        sel = ecore == c
        dslot = perm[dst[sel]]
        sg = grow[src[sel]]
        buc = np.searchsorted(QROWBASE, sg, side="right") - 1
        blk = dslot // P
        o = np.lexsort((dslot, buc, blk))
        dslot, sg, buc, blk = dslot[o], sg[o], buc[o], blk[o]
        cores_edges.append((dslot, sg, buc, blk))
        np.add.at(cnt_cjb[c], (blk, buc), 1)

    A_jb = cnt_cjb.max(axis=0)  # [NBLK, NBUC]
    for j in range(NBLK):
        if A_jb[j].sum() == 0:
            A_jb[j, 0] = 1  # zero-degree block still produces neigh=0

    # balance blocks into groups of <= GRP within each quarter
    C_j = A_jb.sum(axis=1)
    groups = []
    qgroup_end = []
    for q in range(4):
        blocks = list(range(QBLK[q], QBLK[q + 1]))
        ngroups = math.ceil(len(blocks) / GRP)
        order = sorted(blocks, key=lambda j: -C_j[j])
        gsum = np.zeros(ngroups, np.int64)
        gcnt = np.zeros(ngroups, np.int64)
        qgroups = [[] for _ in range(ngroups)]
        for j in order:
            cand = [g for g in range(ngroups) if gcnt[g] < GRP]
            g = min(cand, key=lambda t: gsum[t])
            qgroups[g].append(int(j))
            gsum[g] += C_j[j]
            gcnt[g] += 1
        groups.extend(sorted(g) for g in qgroups)
        qgroup_end.append(len(groups) - 1)
    NG = len(groups)

    # stream layout: for g, for b: packed segment of the group's blocks
    off_jb = np.zeros((NBLK, NBUC), np.int64)
    calls = []  # per group: list of (b, ch0, nch, ni)
    seg_of = {}
    pos = 0
    for g in groups:
        gc = []
        for b in range(NBUC):
            seg = 0
            for j in g:
                seg_of[(j, b)] = (pos, seg)
                off_jb[j, b] = pos * P + seg
                seg += int(A_jb[j, b])
            if seg > 0:
                nch = math.ceil(seg / P)
                gc.append((b, pos, nch, seg))
                pos += nch
        calls.append(gc)
    NCH = pos
    NIDX = NCH * P

    # (chunk, block) matmul pairs and one-hot columns, block-major 8-aligned
    blockmm = {j: [] for j in range(NBLK)}
    dcol0 = np.zeros(NBLK, np.int64)
    dpos = 0
    for j in range(NBLK):
        dcol0[j] = dpos
        i = 0
        for b in range(NBUC):
            if A_jb[j, b] == 0:
                continue
            ch0, so = seg_of[(j, b)]
            lo = ch0 * P + so
            hi = lo + int(A_jb[j, b])
            c_lo, c_hi = lo // P, (hi - 1) // P
            for sp in range(c_lo, c_hi + 1):
                blockmm[j].append((b, sp, dpos + i))
                i += 1
        dpos += 8 * math.ceil(i / 8)
    NCHD = dpos

    # layer-0 c8 sub-tiles: per group, 2 subs of <=2 blocks (DoubleRow
    # interleaved layout [P, VTH, 2, 2*P])
    subblocks = []  # per sub: list of blocks (len<=2)
    sub_of_group = []  # per group: (sub0, sub1)
    for g in groups:
        s0 = len(subblocks)
        subblocks.append(g[0:2])
        subblocks.append(g[2:4])
        sub_of_group.append((s0, s0 + 1))
    NSUB = len(subblocks)

    per_core = []
    for c in range(M):
        dslot, sg, buc, blk = cores_edges[c]
        sel = ecore == c
        cnt = np.zeros((VP, SLOTS), np.int16)
        np.add.at(cnt, (x[src[sel]], perm[dst[sel]]), 1)
        # [VTH, 2, P, NBLK, P] view: vocab row (2*th+i)*P + p
        cnt5 = cnt.reshape(VTH, 2, P, NBLK, P)
        c8 = np.zeros((NSUB, P, VTH, 2, 2 * P), np.float32)
        for s, bl in enumerate(subblocks):
            for w, j in enumerate(bl):
                c8[s, :, :, :, w * P : (w + 1) * P] = cnt5[:, :, :, j, :].transpose(
                    2, 0, 1, 3
                )
        c8 = c8.reshape(NSUB, P, VTH * 2 * 2 * P).astype(ml_dtypes.float8_e4m3fn)

        # rank within (block, bucket)
        flat = (blk * NBUC + buc).astype(np.int64)
        cnts = cnt_cjb[c].reshape(-1)
        st = np.zeros(NBLK * NBUC, np.int64)
        st[1:] = np.cumsum(cnts)[:-1]
        rank = np.arange(len(dslot)) - st[flat]
        spos = off_jb[blk, buc] + rank
        idxs = np.zeros(NIDX, np.int16)
        idxs[spos] = (sg - QROWBASE[buc]).astype(np.int16)
        # NOTE: do NOT mark trailing pad slots as -1 (the gather kernel's
        # trailing-negative trim) — it deadlocks the device even when
        # num_idxs stays >= 1 after trimming (tested twice).

        dstloc = np.full((P, NCHD), 255.0, ml_dtypes.bfloat16)
        ech = spos // P
        first_ch = off_jb[blk, buc] // P
        # dc: walk each (j,b) range; dcol0[j] + offset within block's list
        # recompute per-edge dst col via blockmm structure
        dc = np.zeros(len(dslot), np.int64)
        # per (j, b): columns assigned consecutively from (j,b) range start
        colbase = {}
        for j in range(NBLK):
            for (b, sp, dcol) in blockmm[j]:
                colbase.setdefault((j, b), dcol)
        cb_lookup = np.zeros((NBLK, NBUC), np.int64)
        fc_lookup = np.zeros((NBLK, NBUC), np.int64)
        for (j, b), dcol in colbase.items():
            cb_lookup[j, b] = dcol
            fc_lookup[j, b] = off_jb[j, b] // P
        dc = cb_lookup[blk, buc] + (ech - fc_lookup[blk, buc])
        dstloc[spos % P, dc] = (dslot % P).astype(np.float32)

        idx16 = idxs.reshape(NIDX // 16, 16).T.copy()
        idx_full = np.tile(idx16, (8, 1))

        lo = c * NPC
        invd = 1.0 / np.maximum(deg, 1.0)
        nodes = np.arange(lo, min((c + 1) * NPC, N))
        node_of_slot = np.full(SLOTS, -1, np.int64)
        node_of_slot[perm[nodes]] = nodes
        invd_slot = np.ones(SLOTS, np.float32)
        real = node_of_slot >= 0
        invd_slot[real] = invd[node_of_slot[real]].astype(np.float32)
        # grouped: invd_g[p, g*GRP + bi] = invd_slot[j*P + p]
        invd_g = np.ones((P, NG * GRP), np.float32)
        for g, bl in enumerate(groups):
            for bi, j in enumerate(bl):
                invd_g[:, g * GRP + bi] = invd_slot[j * P : (j + 1) * P]

        x_slot = np.zeros(SLOTS, np.int64)
        x_slot[perm[nodes]] = x[nodes]
        xg = x_slot.astype(np.int16)
        xg16 = np.tile(xg.reshape(SLOTS // 16, 16).T.copy(), (8, 1))

        per_core.append(
            {
                "gidx": idx_full,
                "dstloc": dstloc,
                "invd": invd_g,
                "invd0": invd_g / EMB_SCALE,
                "xgidx": xg16,
                "c8": c8,
            }
        )

    gslot = core_of_node * SLOTS + perm

    meta = {
        "NPC": NPC,
        "SLOTS": SLOTS,
        "NBLK": NBLK,
        "TBL": TBL,
        "QBLK": QBLK,
        "QSTART": QSTART,
        "QNS": QNS,
        "QROWBASE": QROWBASE,
        "BUCV": BUCV,
        "groups": groups,
        "qgroup_end": qgroup_end,
        "calls": calls,
        "blockmm": blockmm,
        "dcol0": dcol0,
        "NCH": NCH,
        "NCHD": NCHD,
        "NIDX": NIDX,
        "NG": NG,
        "NSUB": NSUB,
        "sub_of_group": sub_of_group,
        "gslot": gslot,
    }
    return per_core, meta


def _build_program(meta, V, L, single_core=False):
    SLOTS, NBLK, TBL = meta["SLOTS"], meta["NBLK"], meta["TBL"]
    QBLK, QSTART, QNS = meta["QBLK"], meta["QSTART"], meta["QNS"]
    QROWBASE, BUCV = meta["QROWBASE"], meta["BUCV"]
    groups, qgroup_end = meta["groups"], meta["qgroup_end"]
    calls, blockmm, dcol0 = meta["calls"], meta["blockmm"], meta["dcol0"]
    NCH, NCHD, NIDX, NG = meta["NCH"], meta["NCHD"], meta["NIDX"], meta["NG"]
    NSUB, sub_of_group = meta["NSUB"], meta["sub_of_group"]
    CBMAX = max(nch for gc in calls for (_, _, nch, _) in gc)

    f32, f32r, bf16 = mybir.dt.float32, mybir.dt.float32r, mybir.dt.bfloat16
    i16, f8 = mybir.dt.int16, mybir.dt.float8e4

    nc = bacc.Bacc(
        "TRN2",
        target_bir_lowering=False,
        debug=False,
        enable_asserts=False,
        num_devices=1 if single_core else M,
        num_swdge_queues=NQ,
    )

    gidx_d = nc.dram_tensor("gidx", [P, NIDX // 16], i16, kind="ExternalInput")
    dstloc_d = nc.dram_tensor("dstloc", [P, NCHD], bf16, kind="ExternalInput")
    invd_d = nc.dram_tensor("invd", [P, NG * GRP], f32, kind="ExternalInput")
    invd0_d = nc.dram_tensor("invd0", [P, NG * GRP], f32, kind="ExternalInput")
    emb8_d = nc.dram_tensor("emb8", [P, VT * D], f8, kind="ExternalInput")
    c8_d = nc.dram_tensor("c8", [NSUB, P, VTH * 2 * 2 * P], f8, kind="ExternalInput")
    xgidx_d = nc.dram_tensor("xgidx", [P, SLOTS // 16], i16, kind="ExternalInput")
    emb16_d = nc.dram_tensor("emb16", [V, D], bf16, kind="ExternalInput")
    ws_d = nc.dram_tensor("ws", [L, D, D], f32, kind="ExternalInput")
    wn_d = nc.dram_tensor("wn", [L, D, D], f32, kind="ExternalInput")
    bias_d = nc.dram_tensor("bias", [L, D], f32, kind="ExternalInput")
    hout_d = nc.dram_tensor("hout", [SLOTS, D], bf16, kind="ExternalOutput")

    h_shard = nc.dram_tensor("h_shard", [SLOTS, D], bf16, kind="Internal")
    h_full_t = [
        nc.dram_tensor(
            f"h_full{t}", [TBL, D], bf16, kind="Internal", addr_space="Shared"
        )
        for t in range(2)
    ]

    rg = [list(range(M))]
    qrr = [0]

    with tile.TileContext(nc) as tc:
        with (
            tc.tile_pool(name="const", bufs=1) as cpool,
            tc.tile_pool(name="state", bufs=1) as spool,
            tc.tile_pool(name="gath", bufs=12) as gpool,
            tc.tile_pool(name="oh", bufs=7) as ohpool,
            tc.tile_pool(name="c8", bufs=2) as c8pool,
            tc.tile_pool(name="fm", bufs=2) as fmpool,
            tc.tile_pool(name="small", bufs=1) as smpool,
            tc.tile_pool(name="ps_a", bufs=2, space="PSUM") as ps_a,
            tc.tile_pool(name="ps_t", bufs=2, space="PSUM") as ps_t,
            tc.tile_pool(name="ps_d", bufs=2, space="PSUM") as ps_d,
        ):
            # ---- constants ----
            ident_f = cpool.tile([P, P], f32, tag="ident_f")
            make_identity(nc, ident_f[:])
            ident_h = cpool.tile([P, P], bf16, tag="ident_h")
            make_identity(nc, ident_h[:])
            eps_sb = cpool.tile([P, 1], f32, tag="eps")
            nc.vector.memset(eps_sb[:], 1e-24)
            iota8 = cpool.tile([P, 8 * P], bf16, tag="iota8")
            nc.gpsimd.iota(
                iota8[:].rearrange("p (r c) -> p r c", c=P),
                pattern=[[0, 8], [1, P]],
                base=0,
                channel_multiplier=0,
                allow_small_or_imprecise_dtypes=True,
            )

            gidx_sb = cpool.tile([P, NIDX // 16], i16, tag="gidx")
            nc.sync.dma_start(gidx_sb[:], gidx_d[:, :])
            dstloc_sb = cpool.tile([P, NCHD], bf16, tag="dstloc")
            nc.sync.dma_start(dstloc_sb[:], dstloc_d[:, :])
            invd_sb = cpool.tile([P, NG * GRP], f32, tag="invd")
            nc.sync.dma_start(invd_sb[:], invd_d[:, :])
            invd0_sb = cpool.tile([P, NG * GRP], f32, tag="invd0")
            nc.sync.dma_start(invd0_sb[:], invd0_d[:, :])
            emb8_sb = cpool.tile([P, VT * D], f8, tag="emb8")
            nc.sync.dma_start(emb8_sb[:], emb8_d[:, :])
            xg_sb = cpool.tile([P, SLOTS // 16], i16, tag="xgidx")
            nc.sync.dma_start(xg_sb[:], xgidx_d[:, :])

            w_sb = []
            for l in range(L):
                wsf = cpool.tile([P, D], f32, tag=f"wsf{l}")
                wnf = cpool.tile([P, D], f32, tag=f"wnf{l}")
                nc.sync.dma_start(wsf[:], ws_d[l, :, :])
                nc.sync.dma_start(wnf[:], wn_d[l, :, :])
                ws = cpool.tile([P, D], f32r, tag=f"ws{l}")
                wn = cpool.tile([P, D], f32r, tag=f"wn{l}")
                nc.scalar.copy(ws[:], wsf[:])
                nc.scalar.copy(wn[:], wnf[:])
                w_sb.append((ws, wn))
            b_sb = cpool.tile([P, L], f32, tag="bias")
            for l in range(L):
                nc.sync.dma_start(b_sb[:, l : l + 1], bias_d[l, :, None])

            # ---- embedding lookup: 4 dma_gather calls, one per queue ----
            e_sb = spool.tile([P, NBLK * D], bf16, tag="e")
            ev = e_sb[:].rearrange("p (j f) -> p j f", f=D)
            jsplit = [0, 25, 50, 75, NBLK]
            for qi in range(4):
                j0, j1 = jsplit[qi], jsplit[qi + 1]
                ni = (j1 - j0) * P
                nc.gpsimd.dma_gather(
                    ev[:, j0:j1, :],
                    emb16_d[:, :],
                    xg_sb[:, j0 * 8 : j1 * 8],
                    ni,
                    ni,
                    D,
                    single_packet=False,
                    queue_num=qi % NQ,
                )

            h_sb = spool.tile([P, NBLK * D], bf16, tag="h")

            # zero-init gather ring buffers (NaN * 0 = NaN safety)
            for _ in range(12):
                gz = gpool.tile([P, CBMAX, D], bf16, tag="gath")
                nc.vector.memset(gz[:, :, :], 0.0)

            shard_v = h_shard.ap().rearrange("(j p) f -> p j f", p=P)

            def store_q(q, tab):
                j0, j1 = int(QBLK[q]), int(QBLK[q + 1])
                sv = h_sb[:, j0 * D : j1 * D].rearrange("p (j f) -> p j f", f=D)
                # scalar (ACT) HWDGE ring: the sync ring is clogged with
                # WAR-throttled c8 loads in layer 0, which would delay the
                # store and hence the collective trigger by ~100us.  The
                # store has no WAR wait, so it can't head-of-line block ACT.
                nc.scalar.dma_start(out=shard_v[:, j0:j1, :], in_=sv)
                if single_core:
                    return
                nc.gpsimd.collective_compute(
                    "AllGather",
                    mybir.AluOpType.bypass,
                    replica_groups=rg,
                    ins=[h_shard[int(QSTART[q]) : int(QSTART[q] + QNS[q]), :]],
                    outs=[
                        tab[
                            int(QROWBASE[q]) : int(QROWBASE[q] + M * QNS[q]), :
                        ]
                    ],
                )

            qend_of = {qgroup_end[q]: q for q in range(4)}

            gtiles = {}  # (layer, gi) -> {bucket: (tile, ch0)}

            def issue(li, gi, buckets):
                tab_in = h_full_t[li % 2]
                gt_map = gtiles.setdefault((li, gi), {})
                for (b, ch0, nch, ni) in calls[gi]:
                    if b not in buckets:
                        continue
                    gt = gpool.tile([P, CBMAX, D], bf16, tag="gath")
                    nc.gpsimd.dma_gather(
                        gt[:, 0:nch, :],
                        tab_in[int(QROWBASE[b]) : int(QROWBASE[b] + BUCV[b]), :],
                        gidx_sb[:, ch0 * 8 : (ch0 + nch) * 8],
                        ni,
                        ni,
                        D,
                        single_packet=False,
                        queue_num=qrr[0] % NQ,
                    )
                    qrr[0] += 1
                    gt_map[b] = (gt, ch0)

            # ---- layers ----
            for l in range(L):
                cur = e_sb if l == 0 else h_sb
                h_full = h_full_t[l % 2]
                tab_out = h_full_t[(l + 1) % 2]
                ws, wn = w_sb[l]

                if l > 0:
                    # groups 0/1 b0/b1 were hoisted into the previous
                    # layer; their b2 calls run here (they SEQ-wait on the
                    # Q2 collective, so hoisting them would block Pool)
                    issue(l, 0, (2,))
                    issue(l, 1, (2,))
                    issue(l, 2, (0, 1, 2))
                    issue(l, 0, (3,))
                    issue(l, 1, (3,))

                post = None
                for gi, grp in enumerate(groups):
                    gw = len(grp)
                    if (gi - 1) in qend_of and post is not None:
                        # run the deferred post + quarter store/collective
                        # BEFORE this iteration's gather issues so the
                        # trigger isn't queued behind ring-stalled DGE
                        post()
                        post = None
                        if l < L - 1:
                            store_q(qend_of[gi - 1], tab_out)
                    if l > 0:
                        # order matters for gather-ring WAR safety:
                        # b3(gi+2) must precede b012(gi+3); the 2-group
                        # bucket-3 lead hides the gather latency that
                        # otherwise stalls every group's chain tail
                        if gi + 2 < NG:
                            issue(l, gi + 2, (3,))
                        if gi + 3 < NG:
                            issue(l, gi + 3, (0, 1, 2))
                    pa4 = ps_a.tile([P, GRP * P], f32, tag="pa")
                    if l == 0:
                        # layer 0: neigh-sum = emb8^T @ count-matrix via
                        # fp8 DoubleRow (2 vocab k-tiles per matmul)
                        ev8 = emb8_sb[:].rearrange(
                            "p (t i f) -> p t i f", t=VTH, i=2
                        )
                        for si in range(2):
                            sidx = sub_of_group[gi][si]
                            cs = c8pool.tile(
                                [P, VTH * 2 * 2 * P], f8, tag="c8"
                            )
                            # sync ring only (the ACT ring carries the
                            # quarter stores); split each tile in quarters
                            # so the matmul chain starts on the first one
                            QB = VTH * 2 * 2 * P // 4
                            for ci in range(4):
                                nc.sync.dma_start(
                                    cs[:, ci * QB : (ci + 1) * QB],
                                    c8_d[sidx, :, ci * QB : (ci + 1) * QB],
                                )
                            csv = cs[:].rearrange(
                                "p (t i w) -> p t i w", t=VTH, i=2
                            )
                            for th in range(VTH):
                                nc.tensor.matmul(
                                    pa4[:, si * 2 * P : (si + 1) * 2 * P],
                                    ev8[:, th, :, :],
                                    csv[:, th, :, :],
                                    start=(th == 0),
                                    stop=(th == VTH - 1),
                                    perf_mode=mybir.MatmulPerfMode.DoubleRow,
                                )
                    else:
                        for bi, j in enumerate(grp):
                            mms = blockmm[j]
                            nmm = len(mms)
                            noct = (nmm + 7) // 8
                            d0 = int(dcol0[j])
                            ohqs = []
                            for t in range(noct):
                                ohq = ohpool.tile([P, 8 * P], bf16, tag="oh")
                                s = d0 + 8 * t
                                nc.vector.tensor_tensor(
                                    out=ohq[:].rearrange(
                                        "p (r c) -> p r c", c=P
                                    ),
                                    in0=dstloc_sb[
                                        :, s : s + 8
                                    ].to_broadcast([P, 8, P]),
                                    in1=iota8[:].rearrange(
                                        "p (r c) -> p r c", c=P
                                    ),
                                    op=mybir.AluOpType.is_equal,
                                )
                                ohqs.append(ohq)
                            for ci, (b, sp, dc) in enumerate(mms):
                                gt, ch0 = gtiles[(l, gi)][b]
                                q8, r8 = divmod(dc - d0, 8)
                                nc.tensor.matmul(
                                    pa4[:, bi * P : (bi + 1) * P],
                                    gt[:, sp - ch0, :],
                                    ohqs[q8][:, r8 * P : (r8 + 1) * P],
                                    start=(ci == 0),
                                    stop=(ci == nmm - 1),
                                )
                    nfm = fmpool.tile([P, GRP * D], f32r, tag="nfm", bufs=1)
                    # layer 0 is ACT-heavy; route its PSUM copies to DVE
                    cp_eng = nc.vector.tensor_copy if l == 0 else nc.scalar.copy
                    cp_eng(nfm[:, 0 : gw * D], pa4[:, 0 : gw * P])
                    # self path: transpose cur blocks to feature-major
                    pt4 = ps_t.tile([P, GRP * P], bf16, tag="pt")
                    for bi, j in enumerate(grp):
                        nc.tensor.transpose(
                            pt4[:, bi * P : (bi + 1) * P],
                            cur[:, j * D : (j + 1) * D],
                            ident_h[:],
                        )
                    hfm = fmpool.tile([P, GRP * D], f32r, tag="hfm", bufs=1)
                    nc.scalar.copy(hfm[:, 0 : gw * D], pt4[:, 0 : gw * P])

                    d_ps = ps_d.tile([P, 2 * GRP * D], f32, tag="d")
                    pdS = d_ps[:, 0 : GRP * D]
                    pdN = d_ps[:, GRP * D : 2 * GRP * D]
                    nc.tensor.matmul(
                        pdS[:, 0 : gw * D],
                        ws[:],
                        hfm[:, 0 : gw * D],
                        start=True,
                        stop=True,
                    )
                    nc.tensor.matmul(
                        pdN[:, 0 : gw * D],
                        wn[:],
                        nfm[:, 0 : gw * D],
                        start=True,
                        stop=True,
                    )
                    hbias = fmpool.tile([P, GRP * D], f32, tag="hbias")
                    nc.scalar.activation(
                        hbias[:, 0 : gw * D],
                        pdS[:, 0 : gw * D],
                        mybir.ActivationFunctionType.Identity,
                        bias=b_sb[:, l : l + 1],
                    )
                    nden = fmpool.tile([P, GRP * D], f32, tag="nden")
                    cp_eng(nden[:, 0 : gw * D], pdN[:, 0 : gw * D])

                    def make_post(gi=gi, grp=grp, hbias=hbias, nden=nden, l=l):
                        def run():
                            gw = len(grp)
                            nm4 = ps_d.tile([P, 2 * GRP * D], f32, tag="d")
                            pnS = nm4[:, 0 : GRP * P]
                            pnN = nm4[:, GRP * P : 2 * GRP * P]
                            for bi in range(gw):
                                nc.tensor.transpose(
                                    pnS[:, bi * P : (bi + 1) * P],
                                    hbias[:, bi * D : (bi + 1) * D],
                                    ident_f[:],
                                )
                                nc.tensor.transpose(
                                    pnN[:, bi * P : (bi + 1) * P],
                                    nden[:, bi * D : (bi + 1) * D],
                                    ident_f[:],
                                )
                            inv_l = invd0_sb if l == 0 else invd_sb
                            tn4 = smpool.tile([P, GRP * P], f32, tag="tn4")
                            for bi in range(gw):
                                nc.scalar.activation(
                                    tn4[:, bi * P : (bi + 1) * P],
                                    pnN[:, bi * P : (bi + 1) * P],
                                    mybir.ActivationFunctionType.Identity,
                                    scale=inv_l[
                                        :, gi * GRP + bi : gi * GRP + bi + 1
                                    ],
                                )
                            hp4 = smpool.tile([P, GRP * P], f32, tag="hp4")
                            nc.vector.tensor_tensor(
                                out=hp4[:, 0 : gw * P],
                                in0=pnS[:, 0 : gw * P],
                                in1=tn4[:, 0 : gw * P],
                                op=mybir.AluOpType.add,
                            )
                            # relu into tn4 (dead after the add)
                            nc.scalar.activation(
                                tn4[:, 0 : gw * P],
                                hp4[:, 0 : gw * P],
                                mybir.ActivationFunctionType.Relu,
                            )
                            ss4 = smpool.tile([P, GRP], f32, tag="ss4")
                            for bi in range(gw):
                                # squares land in hp4 (dead); only the
                                # accumulated sum-of-squares is consumed
                                nc.scalar.activation(
                                    hp4[:, bi * P : (bi + 1) * P],
                                    tn4[:, bi * P : (bi + 1) * P],
                                    mybir.ActivationFunctionType.Square,
                                    accum_out=ss4[:, bi : bi + 1],
                                )
                            nrm4 = smpool.tile([P, GRP], f32, tag="nrm4")
                            # sqrt(ss + 1e-24) == the 1e-12 norm floor
                            nc.scalar.activation(
                                nrm4[:, 0:gw],
                                ss4[:, 0:gw],
                                mybir.ActivationFunctionType.Sqrt,
                                bias=eps_sb[:, 0:1],
                            )
                            inv4 = smpool.tile([P, GRP], f32, tag="inv4")
                            nc.vector.reciprocal(inv4[:, 0:gw], nrm4[:, 0:gw])
                            ht4 = smpool.tile([P, GRP * P], f32, tag="ht4")
                            for bi in range(gw):
                                nc.scalar.activation(
                                    ht4[:, bi * P : (bi + 1) * P],
                                    tn4[:, bi * P : (bi + 1) * P],
                                    mybir.ActivationFunctionType.Identity,
                                    scale=inv4[:, bi : bi + 1],
                                )
                            for bi, j in enumerate(grp):
                                nc.vector.tensor_tensor(
                                    out=h_sb[:, j * D : (j + 1) * D],
                                    in0=ht4[:, bi * P : (bi + 1) * P],
                                    in1=e_sb[:, j * D : (j + 1) * D],
                                    op=mybir.AluOpType.add,
                                )
                        return run

                    if post is not None:
                        post()
                    post = make_post()
                post()
                if l < L - 1:
                    store_q(3, tab_out)
                    # hoist next layer's first b0/b1 prefetches here: after
                    # the Q3 collective trigger (so it fires promptly) and
                    # limited to buckets whose collectives are long done so
                    # the hoist never SEQ-blocks the Pool queue
                    issue(l + 1, 0, (0, 1))
                    issue(l + 1, 1, (0, 1))

            hout_v = hout_d.ap().rearrange("(j p) f -> p j f", p=P)
            h_v = h_sb[:].rearrange("p (j f) -> p j f", f=D)
            nc.sync.dma_start(hout_v, h_v)

    nc.compile()
    return nc


def kernel(x, src, dst, emb, Ws, Wn, b, _trace=False):
    x = np.asarray(x)
    src = np.asarray(src)
    dst = np.asarray(dst)
    emb = np.ascontiguousarray(np.asarray(emb, dtype=np.float32))
    Ws = np.ascontiguousarray(np.asarray(Ws, dtype=np.float32))
    Wn = np.ascontiguousarray(np.asarray(Wn, dtype=np.float32))
    b = np.ascontiguousarray(np.asarray(b, dtype=np.float32))
    N = x.shape[0]
    V, _ = emb.shape
    L = Ws.shape[0]

    per_core, meta = _host_prep(x, src, dst, N)
    nc = _build_program(meta, V, L)

    # emb, upscaled for fp8, DoubleRow-interleaved SBUF layout:
    # emb8s[p, (th*2 + i)*D + f] = (emb * EMB_SCALE)[(2*th+i)*P + p, f]
    embp = np.zeros((VP, D), np.float32)
    embp[:V] = emb * EMB_SCALE
    emb8s = np.ascontiguousarray(
        embp.reshape(VT, P, D).transpose(1, 0, 2).reshape(P, VT * D)
    ).astype(ml_dtypes.float8_e4m3fn)
    emb16 = np.ascontiguousarray(emb.astype(ml_dtypes.bfloat16))

    in_maps = []
    for c in range(M):
        pc = per_core[c]
        in_maps.append(
            {
                "gidx": np.ascontiguousarray(pc["gidx"]),
                "dstloc": np.ascontiguousarray(pc["dstloc"]),
                "invd": np.ascontiguousarray(pc["invd"]),
                "invd0": np.ascontiguousarray(pc["invd0"]),
                "xgidx": np.ascontiguousarray(pc["xgidx"]),
                "c8": np.ascontiguousarray(pc["c8"]),
                "emb8": emb8s,
                "emb16": emb16,
                "ws": Ws,
                "wn": Wn,
                "bias": b,
            }
        )

    res = bass_utils.run_bass_kernel_spmd(
        nc, in_maps, core_ids=list(range(M)), trace=_trace
    )
    global LAST_EXEC_NS
    LAST_EXEC_NS = res.exec_time_ns
    outs = [np.asarray(r["hout"], dtype=np.float32) for r in res.results]
    big = np.concatenate(outs, axis=0)
    return big[meta["gslot"]]
